# revision 4
# baseline (speedup 1.0000x reference)
"""Trainium2 Bass kernel for CustomTaylorLayer.

Computes out[b, j] = sum_{i,k} coef[j, i, k] * tanh(x[b, i] * r)^k
for x:[8192,1024], coef:[1024,1024,8], r scalar.

Strategy: data-parallel over the batch across 8 NeuronCores (1024 rows
per core). The k=0 term is an exact host-side column sum folded in as a
per-partition bias at PSUM flush. k=1,2 run as bf16 matmuls. k=3..7 run
as fp8(e4m3) DoubleRow matmuls (2 weights/PE cell, 256-wide contraction
per issue, ~1.8x bf16 ALU rate). To keep fp8 rounding error inside the
2e-2 budget, the device feeds variance-reduced channels
  g_k = t^k - lam_k * t^(1 or 2)
(least-squares projections of high powers onto the exactly-carried bf16
channels); the lam_k * W_k parts are folded into the bf16 weights on the
host, so the reconstruction is exact and only the small residuals see
fp8 quantization (measured rel err ~9e-3 vs 2.8e-2 for naive fp8).
All 7 k-terms of an output tile accumulate into a single PSUM bank
(8 banks = 8 j-tiles in flight per batch-half); one tensor_scalar flush
per tile adds the k=0 bias and stages the DMA out.
"""

import numpy as np
import ml_dtypes
from contextlib import ExitStack

B, IN, OUT, K = 8192, 1024, 1024, 8
NCORES = 8
BLOC = B // NCORES          # 1024 batch rows per core
NI = IN // 128              # 8 i-tiles
NJ = OUT // 128             # 8 j-tiles
NH = BLOC // 512            # 2 batch halves (PSUM bank = 512 fp32)

# Least-squares projection coefficients of t^k onto t (odd k) / t^2
# (even k) under the empirical distribution of t = tanh(x), x ~ N(0,1).
L3 = np.float32(0.6416379)
L4 = np.float32(0.73815584)
L5 = np.float32(0.47362876)
L6 = np.float32(0.58385575)
L7 = np.float32(0.37462398)

_NC_CACHE = {}


def _build_nc():
    import concourse.bacc as bacc
    import concourse.mybir as mybir
    import concourse.tile as tile

    dt = mybir.dt
    AF = mybir.ActivationFunctionType
    ALU = mybir.AluOpType
    DR = mybir.MatmulPerfMode.DoubleRow
    f32 = dt.float32
    bf16 = dt.bfloat16
    f8 = dt.float8e4

    nc = bacc.Bacc("TRN2", target_bir_lowering=False, debug=False)

    xt_d = nc.dram_tensor("xt", [IN, BLOC], f32, kind="ExternalInput").ap()
    w1_d = nc.dram_tensor("w1", [128, NI, OUT], bf16, kind="ExternalInput").ap()
    w2_d = nc.dram_tensor("w2", [128, NI, OUT], bf16, kind="ExternalInput").ap()
    wg_d = {k: nc.dram_tensor(f"wg{k}", [128, NI, OUT], f8,
                              kind="ExternalInput").ap() for k in range(3, K)}
    s_d = nc.dram_tensor("s", [128, NJ], f32, kind="ExternalInput").ap()
    rng_d = nc.dram_tensor("rng", [1, 1], f32, kind="ExternalInput").ap()
    out_d = nc.dram_tensor("outT", [OUT, BLOC], f32, kind="ExternalOutput").ap()

    with tile.TileContext(nc) as tc, ExitStack() as ctx:
        sb = ctx.enter_context(tc.tile_pool(name="sb", bufs=1))
        xp = ctx.enter_context(tc.tile_pool(name="xp", bufs=3))
        sp = ctx.enter_context(tc.tile_pool(name="sp", bufs=2))
        op = ctx.enter_context(tc.tile_pool(name="op", bufs=4))
        pp = ctx.enter_context(tc.tile_pool(name="pp", bufs=8, space="PSUM"))

        r_col = sb.tile([128, 1], f32, tag="rcol")
        nc.sync.dma_start(r_col[:], rng_d.to_broadcast((128, 1)))
        s_cols = sb.tile([128, NJ], f32, tag="scols")
        nc.sync.dma_start(s_cols[:], s_d[:])

        # Persistent channel tensors, [128, i-tile, batch]
        b1 = sb.tile([128, NI, BLOC], bf16, tag="b1")
        b2 = sb.tile([128, NI, BLOC], bf16, tag="b2")
        g = {k: sb.tile([128, NI, BLOC], f8, tag=f"g{k}", name=f"g{k}")
             for k in range(3, K)}
        # Persistent weights
        w1s = sb.tile([128, NI, OUT], bf16, tag="w1s")
        w2s = sb.tile([128, NI, OUT], bf16, tag="w2s")
        wgs = {k: sb.tile([128, NI, OUT], f8, tag=f"wg{k}s", name=f"wg{k}s")
               for k in range(3, K)}

        ones = sb.tile([128, 512], bf16, tag="ones")
        nc.vector.memset(ones[:], 1.0)
        onesf = sb.tile([128, 1], f32, tag="onesf")
        nc.vector.memset(onesf[:], 1.0)

        # Preload the ACT tanh table before any real data arrives.
        warm = sb.tile([128, 1], f32, tag="warm")
        nc.scalar.activation(warm[:], onesf[:], AF.Tanh)

        # W DMA chunks, round-robined onto the GpSimd queues between
        # elementwise work. w1 is fully issued up front (k=1 needs it
        # within ~4us); the rest trickle in k order.
        def wdma(dst, src, lo, hi):
            nc.gpsimd.dma_start(dst[:, lo:hi, :], src[:, lo:hi, :])
        wdma(w1s, w1_d, 0, 4)
        wdma(w1s, w1_d, 4, NI)
        wq = [(w2s, w2_d, 0, 4), (w2s, w2_d, 4, NI)]
        for k in range(3, K):
            wq.append((wgs[k], wg_d[k], 0, NI))

        # Warm the PE HAM clock gate so real MMs start at full speed.
        wps = pp.tile([128, 512], f32, tag="ps")
        for wv in range(12):
            nc.tensor.matmul(wps[:], ones[:, 0:128], ones[:, 0:512],
                             start=(wv == 0), stop=(wv == 11))

        # Elementwise pipeline: per i-tile, per batch-half chunk.
        for it in range(NI):
            xs = xp.tile([128, 1, BLOC], f32, tag="xs")
            nc.sync.dma_start(xs[:, 0, :], xt_d[it * 128:(it + 1) * 128, :])
            if wq:
                wdma(*wq.pop(0))
            for h in range(NH):
                hs = slice(h * 512, (h + 1) * 512)
                t1 = sp.tile([128, 512], f32, tag="t1")
                t2 = sp.tile([128, 512], f32, tag="t2")
                t4 = sp.tile([128, 512], f32, tag="t4")
                u6 = sp.tile([128, 512], f32, tag="u6")
                v3 = sp.tile([128, 512], f32, tag="v3")
                v4 = sp.tile([128, 512], f32, tag="v4")
                v5 = sp.tile([128, 512], f32, tag="v5")
                v6 = sp.tile([128, 512], f32, tag="v6")
                v7 = sp.tile([128, 512], f32, tag="v7")
                # g_k = t^k - L_k * t^(1|2), factored as (t^m - L_k) * t^n
                # so only Act bias / DVE tensor_scalar and plain muls are
                # needed (Pool rejects TensorScalarPtr).
                nc.scalar.activation(t1[:], xs[:, 0, hs], AF.Tanh,
                                     scale=r_col[:, 0:1])
                nc.gpsimd.tensor_copy(b1[:, it, hs], t1[:])
                nc.vector.tensor_mul(t2[:], t1[:], t1[:])
                nc.gpsimd.tensor_copy(b2[:, it, hs], t2[:])
                nc.scalar.activation(v3[:], t2[:], AF.Copy, bias=float(-L3))
                nc.gpsimd.tensor_mul(g[3][:, it, hs], v3[:], t1[:])
                nc.vector.tensor_mul(t4[:], t2[:], t2[:])
                nc.vector.tensor_scalar_add(v4[:], t2[:], float(-L4))
                nc.gpsimd.tensor_mul(g[4][:, it, hs], v4[:], t2[:])
                nc.vector.tensor_mul(u6[:], t4[:], t2[:])
                nc.scalar.activation(v5[:], t4[:], AF.Copy, bias=float(-L5))
                nc.vector.tensor_mul(g[5][:, it, hs], v5[:], t1[:])
                nc.scalar.activation(v6[:], t4[:], AF.Copy, bias=float(-L6))
                nc.vector.tensor_mul(g[6][:, it, hs], v6[:], t2[:])
                nc.vector.tensor_scalar_add(v7[:], u6[:], float(-L7))
                nc.gpsimd.tensor_mul(g[7][:, it, hs], v7[:], t1[:])
        for q in wq:
            wdma(*q)

        # PE: per batch-half, all 8 j-tiles accumulate k=1..7 in PSUM.
        for h in range(NH):
            hs = slice(h * 512, (h + 1) * 512)
            pss = [pp.tile([128, 512], f32, tag="ps", name=f"ps{h}_{j}")
                   for j in range(NJ)]
            # k = 1, 2: bf16. ii-outer so the k=1 matmuls start as soon
            # as the first tanh chunks land.
            for ws, ch, k in ((w1s, b1, 1), (w2s, b2, 2)):
                for ii in range(NI):
                    for j in range(NJ):
                        nc.tensor.matmul(
                            pss[j][:],
                            ws[:, ii, j * 128:(j + 1) * 128],
                            ch[:, ii, hs],
                            start=(k == 1 and ii == 0), stop=False)
            # k = 3..7: fp8 DoubleRow, i-subtile pairs.
            for k in range(3, K):
                for j in range(NJ):
                    for ip in range(NI // 2):
                        nc.tensor.matmul(
                            pss[j][:],
                            wgs[k][:, 2 * ip:2 * ip + 2,
                                   j * 128:(j + 1) * 128],
                            g[k][:, 2 * ip:2 * ip + 2, hs],
                            start=False,
                            stop=(k == K - 1 and ip == NI // 2 - 1),
                            perf_mode=DR)
                    if k == K - 1:
                        ot = op.tile([128, 512], f32, tag="ot")
                        nc.vector.tensor_scalar(
                            ot[:], pss[j][:], s_cols[:, j:j + 1], None,
                            op0=ALU.add)
                        nc.sync.dma_start(
                            out_d[j * 128:(j + 1) * 128, hs], ot[:])

    nc.compile()
    return nc


def _get_nc():
    if "nc" not in _NC_CACHE:
        _NC_CACHE["nc"] = _build_nc()
    return _NC_CACHE["nc"]


def _pack_w(w, dtype):
    # [OUT, IN] -> [128, NI, OUT]
    a = np.ascontiguousarray(w.T.reshape(NI, 128, OUT).transpose(1, 0, 2))
    return a.astype(dtype)


def _make_in_maps(x, tanh_range, coef):
    x = np.asarray(x, dtype=np.float32)
    coef = np.asarray(coef, dtype=np.float64)
    W = {k: coef[:, :, k] for k in range(K)}
    bf = ml_dtypes.bfloat16
    f8 = ml_dtypes.float8_e4m3
    w1 = _pack_w(W[1] + L3 * W[3] + L5 * W[5] + L7 * W[7], bf)
    w2 = _pack_w(W[2] + L4 * W[4] + L6 * W[6], bf)
    wg = {k: _pack_w(W[k], f8) for k in range(3, K)}
    s = W[0].sum(axis=1)                                   # [OUT], exact
    s_cols = np.ascontiguousarray(
        s.reshape(NJ, 128).T).astype(np.float32)           # [128, NJ]
    rng = np.asarray(tanh_range, dtype=np.float32).reshape(1, 1)
    shared = {"w1": w1, "w2": w2, "s": s_cols, "rng": rng}
    for k in range(3, K):
        shared[f"wg{k}"] = wg[k]
    in_maps = []
    for c in range(NCORES):
        xt = np.ascontiguousarray(x[c * BLOC:(c + 1) * BLOC, :].T)
        in_maps.append({"xt": xt, **shared})
    return in_maps


def _ensure_ntff_hook():
    """Register the axon NTFF profile hook if the image's antenv lacks it."""
    import sys
    import types
    try:
        from antenv.axon_hooks import get_axon_ntff_profile_hook  # noqa: F401
        return
    except ImportError:
        pass
    try:
        from trn_agent_boot.trn_boot import _ntff_profile_via_ctypes
        hook = _ntff_profile_via_ctypes("/opt/axon/libaxon_pjrt.so")
    except Exception:
        hook = None
    mod = types.ModuleType("antenv.axon_hooks")
    state = {"hook": hook}
    mod.set_axon_ntff_profile_hook = lambda h: state.__setitem__("hook", h)
    mod.get_axon_ntff_profile_hook = lambda: state["hook"]
    sys.modules["antenv.axon_hooks"] = mod
    import antenv
    antenv.axon_hooks = mod


def _run(x, tanh_range, coef, trace=False):
    from concourse.bass_utils import run_bass_kernel_spmd

    if trace:
        _ensure_ntff_hook()

    nc = _get_nc()
    in_maps = _make_in_maps(x, tanh_range, coef)
    res = run_bass_kernel_spmd(nc, in_maps, core_ids=list(range(NCORES)),
                               trace=trace)
    out = np.empty((B, OUT), dtype=np.float32)
    for c in range(NCORES):
        out[c * BLOC:(c + 1) * BLOC, :] = res.results[c]["outT"].T
    return out, res


def kernel(x, tanh_range, coef):
    out, _ = _run(x, tanh_range, coef, trace=False)
    return out


# revision 8
# speedup vs baseline: 1.6620x; 1.6620x over previous
"""Trainium2 Bass kernel for CustomTaylorLayer.

Computes out[b, j] = sum_{i,k} coef[j, i, k] * tanh(x[b, i] * r)^k
for x:[8192,1024], coef:[1024,1024,8], r scalar.

Strategy: data-parallel over the batch across 8 NeuronCores (1024 rows
per core). The k=0 term is an exact host-side column sum added after the
gather. k=1,2 run as bf16 matmuls on channels b1=tanh, b2=tanh^2.
k=3..7 run as fp8(e4m3) DoubleRow matmuls (2 weights/PE cell, 256-wide
contraction per issue) on factored orthogonal-polynomial channels

  g3 = t(t2-a3), g4 = (t2-a3)(t2-a4), g5 = g3*(t2-a5),
  g6 = g4*(t2-a6), g7 = g5*(t2-a7)          (t2 = tanh^2)

whose variance is 3-10x smaller than the raw powers, so fp8's 3.6%
relative rounding hits much smaller values (measured rel err ~1.0e-2 vs
2.8e-2 for naive fp8). The host exactly re-expresses the power basis in
these channels (triangular polynomial solve) and folds the transform
into the weights. Channels are built with cheap bf16 DVE/Pool ops (one
mul each) and cast to fp8 on the scalar engine; all 7 k-terms of an
output tile accumulate into a single PSUM bank (8 banks = 8 j-tiles in
flight per batch-half) and the result DMAs straight from PSUM to HBM.
"""

import numpy as np
import ml_dtypes
from contextlib import ExitStack

B, IN, OUT, K = 8192, 1024, 1024, 8
NCORES = 8
BLOC = B // NCORES          # 1024 batch rows per core
NI = IN // 128              # 8 i-tiles
NJ = OUT // 128             # 8 j-tiles
NH = BLOC // 512            # 2 batch halves (PSUM bank = 512 fp32)

# Sequential weighted-least-squares roots of the factored channels under
# the empirical distribution of t = tanh(x), x ~ N(0,1).
A3 = 0.641655
A4 = 0.153814
A5 = 0.500317
A6 = 0.705566
A7 = 0.570340
# fp8 cast scales (applied on device in the Act cast, divided out of W).
SC = {3: 1.0, 4: 1.0, 5: 2.0, 6: 2.0, 7: 2.0}

_NC_CACHE = {}


def _build_nc():
    import concourse.bacc as bacc
    import concourse.mybir as mybir
    import concourse.tile as tile

    dt = mybir.dt
    AF = mybir.ActivationFunctionType
    DR = mybir.MatmulPerfMode.DoubleRow
    f32 = dt.float32
    bf16 = dt.bfloat16
    f8 = dt.float8e4

    nc = bacc.Bacc("TRN2", target_bir_lowering=False, debug=False)

    xt_d = nc.dram_tensor("xt", [IN, BLOC], f32, kind="ExternalInput").ap()
    w1_d = nc.dram_tensor("w1", [128, NI, OUT], bf16, kind="ExternalInput").ap()
    w2_d = nc.dram_tensor("w2", [128, NI, OUT], bf16, kind="ExternalInput").ap()
    wg_d = {k: nc.dram_tensor(f"wg{k}", [128, NI, OUT], f8,
                              kind="ExternalInput").ap() for k in range(3, K)}
    rng_d = nc.dram_tensor("rng", [1, 1], f32, kind="ExternalInput").ap()
    out_d = nc.dram_tensor("outT", [OUT, BLOC], f32, kind="ExternalOutput").ap()

    with tile.TileContext(nc) as tc, ExitStack() as ctx:
        sb = ctx.enter_context(tc.tile_pool(name="sb", bufs=1))
        xp = ctx.enter_context(tc.tile_pool(name="xp", bufs=3))
        sp = ctx.enter_context(tc.tile_pool(name="sp", bufs=2))
        op = ctx.enter_context(tc.tile_pool(name="op", bufs=4))
        pp = ctx.enter_context(tc.tile_pool(name="pp", bufs=8, space="PSUM"))

        r_col = sb.tile([128, 1], f32, tag="rcol")
        nc.sync.dma_start(r_col[:], rng_d.to_broadcast((128, 1)))

        # Persistent channel tensors, [128, i-tile, batch]
        b1 = sb.tile([128, NI, BLOC], bf16, tag="b1")
        b2 = sb.tile([128, NI, BLOC], bf16, tag="b2")
        g = {k: sb.tile([128, NI, BLOC], f8, tag=f"g{k}", name=f"g{k}")
             for k in range(3, K)}
        # Persistent weights
        w1s = sb.tile([128, NI, OUT], bf16, tag="w1s")
        w2s = sb.tile([128, NI, OUT], bf16, tag="w2s")
        wgs = {k: sb.tile([128, NI, OUT], f8, tag=f"wg{k}s", name=f"wg{k}s")
               for k in range(3, K)}

        ones = sb.tile([128, 512], bf16, tag="ones")
        nc.vector.memset(ones[:], 1.0)
        onesf = sb.tile([128, 1], f32, tag="onesf")
        nc.vector.memset(onesf[:], 1.0)

        # Preload the ACT tanh table before any real data arrives.
        warm = sb.tile([128, 1], f32, tag="warm")
        nc.scalar.activation(warm[:], onesf[:], AF.Tanh)

        # W DMA chunks: w1 fully up front (k=1 needs it within ~4us), the
        # rest trickled onto the GpSimd queues between elementwise work.
        def wdma(dst, src, lo, hi):
            nc.gpsimd.dma_start(dst[:, lo:hi, :], src[:, lo:hi, :])
        wdma(w1s, w1_d, 0, 4)
        wdma(w1s, w1_d, 4, NI)
        wq = [(w2s, w2_d, 0, 4), (w2s, w2_d, 4, NI)]
        for k in range(3, K):
            wq.append((wgs[k], wg_d[k], 0, NI))

        # Warm the PE HAM clock gate so real MMs start at full speed.
        wps = pp.tile([128, 512], f32, tag="ps")
        for wv in range(12):
            nc.tensor.matmul(wps[:], ones[:, 0:128], ones[:, 0:512],
                             start=(wv == 0), stop=(wv == 11))

        # Elementwise pipeline: per i-tile, per batch-half chunk. All bf16
        # (fast 16-bit DVE/Pool paths); fp8 casts on the Act engine.
        for it in range(NI):
            xs = xp.tile([128, 1, BLOC], f32, tag="xs")
            nc.sync.dma_start(xs[:, 0, :], xt_d[it * 128:(it + 1) * 128, :])
            if wq:
                wdma(*wq.pop(0))
            for h in range(NH):
                hs = slice(h * 512, (h + 1) * 512)
                b1c = b1[:, it, hs]
                b2c = b2[:, it, hs]
                v = {k: sp.tile([128, 512], bf16, tag=f"v{k}", name=f"v{k}")
                     for k in range(3, K)}
                w = {k: sp.tile([128, 512], bf16, tag=f"w{k}", name=f"w{k}")
                     for k in range(3, K)}
                nc.scalar.activation(b1c, xs[:, 0, hs], AF.Tanh,
                                     scale=r_col[:, 0:1])
                nc.vector.tensor_mul(b2c, b1c, b1c)
                for k, a in ((3, A3), (4, A4), (5, A5), (6, A6), (7, A7)):
                    nc.vector.tensor_scalar_add(v[k][:], b2c, float(-a))
                nc.vector.tensor_mul(w[3][:], v[3][:], b1c)
                nc.gpsimd.tensor_mul(w[4][:], v[3][:], v[4][:])
                nc.vector.tensor_mul(w[5][:], w[3][:], v[5][:])
                nc.gpsimd.tensor_mul(w[6][:], w[4][:], v[6][:])
                nc.vector.tensor_mul(w[7][:], w[5][:], v[7][:])
                for k in range(3, K):
                    nc.scalar.activation(g[k][:, it, hs], w[k][:], AF.Copy,
                                         scale=float(SC[k]))
        for q in wq:
            wdma(*q)

        # PE: per batch-half, all 8 j-tiles accumulate k=1..7 in PSUM,
        # then DMA straight from PSUM to HBM.
        for h in range(NH):
            hs = slice(h * 512, (h + 1) * 512)
            pss = [pp.tile([128, 512], f32, tag="ps", name=f"ps{h}_{j}")
                   for j in range(NJ)]
            # k = 1, 2: bf16. ii-outer so the k=1 matmuls start as soon
            # as the first tanh chunks land.
            for ws, ch, k in ((w1s, b1, 1), (w2s, b2, 2)):
                for ii in range(NI):
                    for j in range(NJ):
                        nc.tensor.matmul(
                            pss[j][:],
                            ws[:, ii, j * 128:(j + 1) * 128],
                            ch[:, ii, hs],
                            start=(k == 1 and ii == 0), stop=False)
            # k = 3..7: fp8 DoubleRow, i-subtile pairs.
            for k in range(3, K):
                for j in range(NJ):
                    for ip in range(NI // 2):
                        nc.tensor.matmul(
                            pss[j][:],
                            wgs[k][:, 2 * ip:2 * ip + 2,
                                   j * 128:(j + 1) * 128],
                            g[k][:, 2 * ip:2 * ip + 2, hs],
                            start=False,
                            stop=(k == K - 1 and ip == NI // 2 - 1),
                            perf_mode=DR)
                    if k == K - 1:
                        ot = op.tile([128, 512], f32, tag="ot")
                        nc.scalar.activation(ot[:], pss[j][:], AF.Copy)
                        nc.sync.dma_start(
                            out_d[j * 128:(j + 1) * 128, hs], ot[:])

    nc.compile()
    return nc


def _get_nc():
    if "nc" not in _NC_CACHE:
        _NC_CACHE["nc"] = _build_nc()
    return _NC_CACHE["nc"]


def _channel_polys():
    """Power-basis coefficients of the 7 channels, and the inverse map."""
    import numpy.polynomial.polynomial as P

    def pm(*ps):
        r = np.array([1.0])
        for p in ps:
            r = P.polymul(r, p)
        return r

    q = {k: np.array([-a, 0.0, 1.0]) for k, a in
         ((3, A3), (4, A4), (5, A5), (6, A6), (7, A7))}
    t = np.array([0.0, 1.0])
    CH = {1: t, 2: np.array([0.0, 0.0, 1.0]),
          3: pm(t, q[3]), 4: pm(q[3], q[4]), 5: pm(t, q[3], q[5]),
          6: pm(q[3], q[4], q[6]), 7: pm(t, q[3], q[5], q[7])}
    C = np.zeros((7, 8))
    for m in range(1, 8):
        cc = CH[m]
        C[m - 1, :len(cc)] = cc
    M = C[:, 1:8]                       # channel_m = consts + M @ powers
    consts = C[:, 0]
    Binv = np.linalg.inv(M)             # powers = Binv @ (channels - consts)
    return Binv, consts


def _pack_w(w, dtype):
    # [OUT, IN] -> [128, NI, OUT]
    a = np.ascontiguousarray(w.T.reshape(NI, 128, OUT).transpose(1, 0, 2))
    return a.astype(dtype)


def _make_in_maps(x, tanh_range, coef):
    x = np.asarray(x, dtype=np.float32)
    coef = np.asarray(coef, dtype=np.float64)
    W = {k: coef[:, :, k] for k in range(K)}
    Binv, consts = _channel_polys()
    Wp = {m: sum(W[k] * Binv[k - 1, m - 1] for k in range(1, 8))
          for m in range(1, 8)}
    s = W[0].sum(axis=1)
    for k in range(1, 8):
        cst = sum(Binv[k - 1, m - 1] * consts[m - 1] for m in range(1, 8))
        s -= cst * W[k].sum(axis=1)
    bf = ml_dtypes.bfloat16
    f8 = ml_dtypes.float8_e4m3
    shared = {"w1": _pack_w(Wp[1], bf), "w2": _pack_w(Wp[2], bf),
              "rng": np.asarray(tanh_range, np.float32).reshape(1, 1)}
    for k in range(3, K):
        shared[f"wg{k}"] = _pack_w(Wp[k] / SC[k], f8)
    in_maps = []
    for c in range(NCORES):
        xt = np.ascontiguousarray(x[c * BLOC:(c + 1) * BLOC, :].T)
        in_maps.append({"xt": xt, **shared})
    return in_maps, s.astype(np.float32)


def _ensure_ntff_hook():
    """Register the axon NTFF profile hook if the image's antenv lacks it."""
    import sys
    import types
    try:
        from antenv.axon_hooks import get_axon_ntff_profile_hook  # noqa: F401
        return
    except ImportError:
        pass
    try:
        from trn_agent_boot.trn_boot import _ntff_profile_via_ctypes
        hook = _ntff_profile_via_ctypes("/opt/axon/libaxon_pjrt.so")
    except Exception:
        hook = None
    mod = types.ModuleType("antenv.axon_hooks")
    state = {"hook": hook}
    mod.set_axon_ntff_profile_hook = lambda h: state.__setitem__("hook", h)
    mod.get_axon_ntff_profile_hook = lambda: state["hook"]
    sys.modules["antenv.axon_hooks"] = mod
    import antenv
    antenv.axon_hooks = mod


def _run(x, tanh_range, coef, trace=False):
    from concourse.bass_utils import run_bass_kernel_spmd

    if trace:
        _ensure_ntff_hook()

    nc = _get_nc()
    in_maps, s = _make_in_maps(x, tanh_range, coef)
    res = run_bass_kernel_spmd(nc, in_maps, core_ids=list(range(NCORES)),
                               trace=trace)
    out = np.empty((B, OUT), dtype=np.float32)
    for c in range(NCORES):
        out[c * BLOC:(c + 1) * BLOC, :] = res.results[c]["outT"].T
    out += s[None, :]
    return out, res


def kernel(x, tanh_range, coef):
    out, _ = _run(x, tanh_range, coef, trace=False)
    return out


# revision 9
# speedup vs baseline: 2.1625x; 1.3011x over previous
"""Trainium2 Bass kernel for CustomTaylorLayer.

Computes out[b, j] = sum_{i,k} coef[j, i, k] * tanh(x[b, i] * r)^k
for x:[8192,1024], coef:[1024,1024,8], r scalar.

Strategy: data-parallel over the batch across 8 NeuronCores (1024 rows
per core). The k=0 term is an exact host-side column sum added after the
gather. k=1 runs as bf16 matmuls on b1=tanh. k=2..7 run as fp8(e4m3)
DoubleRow matmuls (2 weights/PE cell, 256-wide contraction per issue) on
variance-reduced channels

  g2 = 2(t2 - mu2),          g3 = t(t2-a3),      g4 = (t2-a3)(t2-a4),
  g5 = g3*(t2-a5),           g6 = g4*(t2-a6),    g7 = g5*(t2-a7)

(t2 = tanh^2; factored orthogonal-ish polynomials whose rms is 3-10x
smaller than the raw powers, so fp8's 3.6% relative rounding hits much
smaller values). The host exactly re-expresses the power basis in these
channels (triangular polynomial solve) and folds the transform into the
weights; constants fold into the k=0 bias. Channels are built with bf16
DVE ops (one mul each, two-pass level order so low-k channels for the
first batch-half are ready before the PE needs them) and cast to fp8 on
the Act engine. All 7 k-terms of an output tile accumulate into a
single PSUM bank (8 banks = 8 j-tiles in flight per batch-half); Act
flushes each bank once to SBUF and the result DMAs out.
"""

import numpy as np
import ml_dtypes
from contextlib import ExitStack

B, IN, OUT, K = 8192, 1024, 1024, 8
NCORES = 8
BLOC = B // NCORES          # 1024 batch rows per core
NI = IN // 128              # 8 i-tiles
NJ = OUT // 128             # 8 j-tiles
NH = BLOC // 512            # 2 batch halves (PSUM bank = 512 fp32)

# Sequential weighted-least-squares roots of the factored channels under
# the empirical distribution of t = tanh(x), x ~ N(0,1), and E[t^2].
A3 = 0.641655
A4 = 0.153814
A5 = 0.500317
A6 = 0.705566
A7 = 0.570340
MU2 = 0.39426075880007483
# fp8 cast scales (applied on device in the Act cast, divided out of W).
SC = {2: 2.0, 3: 1.0, 4: 1.0, 5: 2.0, 6: 2.0, 7: 2.0}

_NC_CACHE = {}


def _build_nc():
    import concourse.bacc as bacc
    import concourse.mybir as mybir
    import concourse.tile as tile

    dt = mybir.dt
    AF = mybir.ActivationFunctionType
    DR = mybir.MatmulPerfMode.DoubleRow
    f32 = dt.float32
    bf16 = dt.bfloat16
    f8 = dt.float8e4

    nc = bacc.Bacc("TRN2", target_bir_lowering=False, debug=False)

    xt_d = nc.dram_tensor("xt", [IN, BLOC], f32, kind="ExternalInput").ap()
    w1_d = nc.dram_tensor("w1", [128, NI, OUT], bf16, kind="ExternalInput").ap()
    wg_d = {k: nc.dram_tensor(f"wg{k}", [128, NI, OUT], f8,
                              kind="ExternalInput").ap() for k in range(2, K)}
    rng_d = nc.dram_tensor("rng", [1, 1], f32, kind="ExternalInput").ap()
    out_d = nc.dram_tensor("outT", [OUT, BLOC], f32, kind="ExternalOutput").ap()

    with tile.TileContext(nc) as tc, ExitStack() as ctx:
        sb = ctx.enter_context(tc.tile_pool(name="sb", bufs=1))
        xp = ctx.enter_context(tc.tile_pool(name="xp", bufs=3))
        sp = ctx.enter_context(tc.tile_pool(name="sp", bufs=2))
        wp = ctx.enter_context(tc.tile_pool(name="wp", bufs=9))
        op = ctx.enter_context(tc.tile_pool(name="op", bufs=3))
        pp = ctx.enter_context(tc.tile_pool(name="pp", bufs=8, space="PSUM"))

        r_col = sb.tile([128, 1], f32, tag="rcol")
        nc.sync.dma_start(r_col[:], rng_d.to_broadcast((128, 1)))

        # Persistent channel tensors, [128, i-tile, batch]
        b1 = sb.tile([128, NI, BLOC], bf16, tag="b1")
        b2 = sb.tile([128, NI, BLOC], bf16, tag="b2")
        g = {k: sb.tile([128, NI, BLOC], f8, tag=f"g{k}", name=f"g{k}")
             for k in range(2, K)}
        # Persistent weights
        w1s = sb.tile([128, NI, OUT], bf16, tag="w1s")
        wgs = {k: sb.tile([128, NI, OUT], f8, tag=f"wg{k}s", name=f"wg{k}s")
               for k in range(2, K)}

        ones = sb.tile([128, 512], bf16, tag="ones")
        nc.vector.memset(ones[:], 1.0)
        onesf = sb.tile([128, 1], f32, tag="onesf")
        nc.vector.memset(onesf[:], 1.0)

        # Preload the ACT tanh table before any real data arrives.
        warm = sb.tile([128, 1], f32, tag="warm")
        nc.scalar.activation(warm[:], onesf[:], AF.Tanh)

        # W DMAs on the GpSimd queues: w1 fully up front (k=1 needs it
        # within ~4us), the rest trickled in k order.
        def wdma(dst, src, lo, hi):
            nc.gpsimd.dma_start(dst[:, lo:hi, :], src[:, lo:hi, :])
        wdma(w1s, w1_d, 0, 4)
        wdma(w1s, w1_d, 4, NI)
        wq = [(wgs[k], wg_d[k], 0, NI) for k in range(2, K)]

        # Warm the PE HAM clock gate so real MMs start at full speed.
        wps = pp.tile([128, 512], f32, tag="ps")
        for wv in range(12):
            nc.tensor.matmul(wps[:], ones[:, 0:128], ones[:, 0:512],
                             start=(wv == 0), stop=(wv == 11))

        # xs DMAs + tanh, h-half major so h=0 channels complete first.
        xss = {}
        for h in range(NH):
            for it in range(NI):
                xs = xp.tile([128, 512], f32, tag="xs", name=f"xs{h}_{it}")
                xss[(h, it)] = xs
                nc.sync.dma_start(
                    xs[:], xt_d[it * 128:(it + 1) * 128,
                                h * 512:(h + 1) * 512])
                if wq and it % 2 == 0:
                    wdma(*wq.pop(0))
        for h in range(NH):
            for it in range(NI):
                nc.scalar.activation(b1[:, it, h * 512:(h + 1) * 512],
                                     xss[(h, it)][:], AF.Tanh,
                                     scale=r_col[:, 0:1])
        for q in wq:
            wdma(*q)

        def emit_dve_a(h, w3s_, w4s_):
            # pass A: b2, v3, w3, v4, w4 per chunk -> enables g2, g3, g4
            hs = slice(h * 512, (h + 1) * 512)
            for it in range(NI):
                b1c, b2c = b1[:, it, hs], b2[:, it, hs]
                v3 = sp.tile([128, 512], bf16, tag="v3")
                v4 = sp.tile([128, 512], bf16, tag="v4")
                w3 = wp.tile([128, 512], bf16, tag="w3", name=f"w3_{h}_{it}")
                w4 = wp.tile([128, 512], bf16, tag="w4", name=f"w4_{h}_{it}")
                w3s_.append(w3)
                w4s_.append(w4)
                nc.vector.tensor_mul(b2c, b1c, b1c)
                nc.vector.tensor_scalar_add(v3[:], b2c, float(-A3))
                nc.vector.tensor_mul(w3[:], v3[:], b1c)
                nc.vector.tensor_scalar_add(v4[:], b2c, float(-A4))
                nc.vector.tensor_mul(w4[:], v3[:], v4[:])

        def emit_dve_b(h, w3s_, w4s_, w57s_):
            # pass B: v5..v7, w5..w7 per chunk -> enables g5, g6, g7
            hs = slice(h * 512, (h + 1) * 512)
            for it in range(NI):
                b1c, b2c = b1[:, it, hs], b2[:, it, hs]
                v5 = sp.tile([128, 512], bf16, tag="v5")
                v6 = sp.tile([128, 512], bf16, tag="v6")
                v7 = sp.tile([128, 512], bf16, tag="v7")
                w5 = sp.tile([128, 512], bf16, tag="w5")
                w6 = sp.tile([128, 512], bf16, tag="w6")
                w7 = sp.tile([128, 512], bf16, tag="w7")
                w57s_.append((w5, w6, w7))
                nc.vector.tensor_scalar_add(v5[:], b2c, float(-A5))
                nc.vector.tensor_mul(w5[:], w3s_[it][:], v5[:])
                nc.vector.tensor_scalar_add(v6[:], b2c, float(-A6))
                nc.vector.tensor_mul(w6[:], w4s_[it][:], v6[:])
                nc.vector.tensor_scalar_add(v7[:], b2c, float(-A7))
                nc.vector.tensor_mul(w7[:], w5[:], v7[:])

        def emit_casts_a(h, w3s_, w4s_):
            hs = slice(h * 512, (h + 1) * 512)
            for it in range(NI):
                nc.scalar.activation(g[2][:, it, hs], b2[:, it, hs], AF.Copy,
                                     scale=float(SC[2]),
                                     bias=float(-SC[2] * MU2))
                nc.scalar.activation(g[3][:, it, hs], w3s_[it][:], AF.Copy,
                                     scale=float(SC[3]))
                nc.scalar.activation(g[4][:, it, hs], w4s_[it][:], AF.Copy,
                                     scale=float(SC[4]))

        def emit_casts_b(h, w57s_):
            hs = slice(h * 512, (h + 1) * 512)
            for it in range(NI):
                w5, w6, w7 = w57s_[it]
                nc.scalar.activation(g[5][:, it, hs], w5[:], AF.Copy,
                                     scale=float(SC[5]))
                nc.scalar.activation(g[6][:, it, hs], w6[:], AF.Copy,
                                     scale=float(SC[6]))
                nc.scalar.activation(g[7][:, it, hs], w7[:], AF.Copy,
                                     scale=float(SC[7]))

        def emit_pe(h, pss):
            hs = slice(h * 512, (h + 1) * 512)
            # k = 1: bf16, ii-outer so matmuls start with the first chunks.
            for ii in range(NI):
                for j in range(NJ):
                    nc.tensor.matmul(
                        pss[j][:], w1s[:, ii, j * 128:(j + 1) * 128],
                        b1[:, ii, hs], start=(ii == 0), stop=False)
            # k = 2..7: fp8 DoubleRow, i-subtile pairs.
            for k in range(2, K):
                for j in range(NJ):
                    for ip in range(NI // 2):
                        nc.tensor.matmul(
                            pss[j][:],
                            wgs[k][:, 2 * ip:2 * ip + 2,
                                   j * 128:(j + 1) * 128],
                            g[k][:, 2 * ip:2 * ip + 2, hs],
                            start=False,
                            stop=(k == K - 1 and ip == NI // 2 - 1),
                            perf_mode=DR)

        def emit_flush(h, pss):
            hs = slice(h * 512, (h + 1) * 512)
            for j in range(NJ):
                ot = op.tile([128, 512], f32, tag="ot")
                nc.scalar.activation(ot[:], pss[j][:], AF.Copy)
                nc.gpsimd.dma_start(out_d[j * 128:(j + 1) * 128, hs], ot[:])

        w3s = {0: [], 1: []}
        w4s = {0: [], 1: []}
        w57s = {0: [], 1: []}
        pss = {h: [pp.tile([128, 512], f32, tag="ps", name=f"ps{h}_{j}")
                   for j in range(NJ)] for h in range(NH)}

        emit_dve_a(0, w3s[0], w4s[0])
        emit_casts_a(0, w3s[0], w4s[0])
        emit_dve_b(0, w3s[0], w4s[0], w57s[0])
        emit_casts_b(0, w57s[0])
        emit_pe(0, pss[0])
        emit_flush(0, pss[0])
        emit_dve_a(1, w3s[1], w4s[1])
        emit_casts_a(1, w3s[1], w4s[1])
        emit_dve_b(1, w3s[1], w4s[1], w57s[1])
        emit_casts_b(1, w57s[1])
        emit_pe(1, pss[1])
        emit_flush(1, pss[1])

    nc.compile()
    return nc


def _get_nc():
    if "nc" not in _NC_CACHE:
        _NC_CACHE["nc"] = _build_nc()
    return _NC_CACHE["nc"]


def _channel_polys():
    """Power-basis coefficients of the 7 channels, and the inverse map."""
    import numpy.polynomial.polynomial as P

    def pm(*ps):
        r = np.array([1.0])
        for p in ps:
            r = P.polymul(r, p)
        return r

    q = {k: np.array([-a, 0.0, 1.0]) for k, a in
         ((3, A3), (4, A4), (5, A5), (6, A6), (7, A7))}
    t = np.array([0.0, 1.0])
    CH = {1: t, 2: np.array([0.0, 0.0, 1.0]),
          3: pm(t, q[3]), 4: pm(q[3], q[4]), 5: pm(t, q[3], q[5]),
          6: pm(q[3], q[4], q[6]), 7: pm(t, q[3], q[5], q[7])}
    C = np.zeros((7, 8))
    for m in range(1, 8):
        cc = CH[m]
        C[m - 1, :len(cc)] = cc
    M = C[:, 1:8]                       # channel_m = consts + M @ powers
    consts = C[:, 0]
    Binv = np.linalg.inv(M)             # powers = Binv @ (channels - consts)
    return Binv, consts


def _pack_w(w, dtype):
    # [OUT, IN] -> [128, NI, OUT]
    a = np.ascontiguousarray(w.T.reshape(NI, 128, OUT).transpose(1, 0, 2))
    return a.astype(dtype)


def _make_in_maps(x, tanh_range, coef):
    x = np.asarray(x, dtype=np.float32)
    coef = np.asarray(coef, dtype=np.float64)
    W = {k: coef[:, :, k] for k in range(K)}
    Binv, consts = _channel_polys()
    Wp = {m: sum(W[k] * Binv[k - 1, m - 1] for k in range(1, 8))
          for m in range(1, 8)}
    s = W[0].sum(axis=1)
    for k in range(1, 8):
        cst = sum(Binv[k - 1, m - 1] * consts[m - 1] for m in range(1, 8))
        s -= cst * W[k].sum(axis=1)
    # channel 2 is fed as SC2*(t2 - MU2): fold the mean term into s.
    s += MU2 * Wp[2].sum(axis=1)
    bf = ml_dtypes.bfloat16
    f8 = ml_dtypes.float8_e4m3
    shared = {"w1": _pack_w(Wp[1], bf),
              "rng": np.asarray(tanh_range, np.float32).reshape(1, 1)}
    for k in range(2, K):
        shared[f"wg{k}"] = _pack_w(Wp[k] / SC[k], f8)
    in_maps = []
    for c in range(NCORES):
        xt = np.ascontiguousarray(x[c * BLOC:(c + 1) * BLOC, :].T)
        in_maps.append({"xt": xt, **shared})
    return in_maps, s.astype(np.float32)


def _ensure_ntff_hook():
    """Register the axon NTFF profile hook if the image's antenv lacks it."""
    import sys
    import types
    try:
        from antenv.axon_hooks import get_axon_ntff_profile_hook  # noqa: F401
        return
    except ImportError:
        pass
    try:
        from trn_agent_boot.trn_boot import _ntff_profile_via_ctypes
        hook = _ntff_profile_via_ctypes("/opt/axon/libaxon_pjrt.so")
    except Exception:
        hook = None
    mod = types.ModuleType("antenv.axon_hooks")
    state = {"hook": hook}
    mod.set_axon_ntff_profile_hook = lambda h: state.__setitem__("hook", h)
    mod.get_axon_ntff_profile_hook = lambda: state["hook"]
    sys.modules["antenv.axon_hooks"] = mod
    import antenv
    antenv.axon_hooks = mod


def _run(x, tanh_range, coef, trace=False):
    from concourse.bass_utils import run_bass_kernel_spmd

    if trace:
        _ensure_ntff_hook()

    nc = _get_nc()
    in_maps, s = _make_in_maps(x, tanh_range, coef)
    res = run_bass_kernel_spmd(nc, in_maps, core_ids=list(range(NCORES)),
                               trace=trace)
    out = np.empty((B, OUT), dtype=np.float32)
    for c in range(NCORES):
        out[c * BLOC:(c + 1) * BLOC, :] = res.results[c]["outT"].T
    out += s[None, :]
    return out, res


def kernel(x, tanh_range, coef):
    out, _ = _run(x, tanh_range, coef, trace=False)
    return out
